# revision 42
# baseline (speedup 1.0000x reference)
"""NetGINE (4-layer GIN message passing) on 8 Trainium2 NeuronCores.

Sharding: nodes/edges sharded by destination across 8 cores (6400 padded node
slots per core). Node table is bf16 pair-packed ([25600, 128]: row r holds
nodes 2r and 2r+1) so each 256B gather record moves 2 nodes; edges are grouped
per dst tile by src PARITY so each 128-edge block reads a fixed half of the
wide gather records. Per layer:
  - bond encoder e = relu(ea @ be1) @ be2 on TensorE (2-group stacked chain,
    pairing (even-block k, odd-block k))
  - h[src] gathered per edge via gpsimd dma_gather, 4 calls per chunk spread
    over all 4 SWDGE queues (8 Q7 cores generate descriptors concurrently)
  - msg = relu(h_src + e); segment-sum via matmuls against a host-precomputed
    fp8 one-hot streamed from HBM, accumulating in PSUM
  - node MLP (bf16) + BN on transposed [64, nodes] tiles
  - AllGather of the [3200, 128] bf16 shard rebuilds the replicated table
Pooling: host-precomputed fp8 graph one-hot against SBUF-resident bf16 node
tiles + AllReduce; head MLP computed redundantly on every core.
"""

import os
import numpy as np
import ml_dtypes

BF16 = np.dtype(ml_dtypes.bfloat16)
FP8 = np.dtype(ml_dtypes.float8_e4m3)

N, E, G, DIM, XF, EF = 50000, 800000, 512, 64, 28, 3
NCORES = 8
NLOC = 6400              # padded node slots per core
NPAD = NCORES * NLOC     # 51200
NPAIR = NPAD // 2        # 25600 pair rows in the bf16 table
TILES = NLOC // 128      # 50
TPC = 4                  # tiles per chunk
NLAYERS = 4
GWIN = 4                 # 128-graph pooling windows


# ---------------------------------------------------------------- host prep --

def _plan_nodes(batch):
    """Assign nodes to per-core padded slots; no 128-slot tile may span a
    128-graph window boundary."""
    slot2node = np.full((NCORES, NLOC), -1, np.int64)
    node2pad = np.full(N, -1, np.int64)
    per_core = N // NCORES  # 6250
    for c in range(NCORES):
        nodes = np.arange(c * per_core, (c + 1) * per_core)
        wins = batch[nodes] // 128
        change = np.nonzero(np.diff(wins))[0] + 1
        bounds = [0] + list(change) + [len(nodes)]
        s = 0
        for i in range(len(bounds) - 1):
            lo, hi = bounds[i], bounds[i + 1]
            if i > 0 and s % 128 != 0:
                s += 128 - (s % 128)
            cnt = hi - lo
            assert s + cnt <= NLOC, "node padding overflow"
            slot2node[c, s:s + cnt] = nodes[lo:hi]
            node2pad[nodes[lo:hi]] = c * NLOC + s + np.arange(cnt)
            s += cnt
    return slot2node, node2pad


def _prep(inputs):
    x = np.asarray(inputs["x"], np.float32)
    edge_attr = np.asarray(inputs["edge_attr"], np.float32)
    edge_index = np.asarray(inputs["edge_index"], np.int64)
    batch = np.asarray(inputs["batch"], np.int64)

    slot2node, node2pad = _plan_nodes(batch)

    src_p = node2pad[edge_index[0]]
    dst_p = node2pad[edge_index[1]]
    core = dst_p // NLOC
    dslot = dst_p % NLOC
    tile_of = dslot // 128
    drel = dslot % 128
    par = src_p % 2            # 0 = even src slot, 1 = odd
    pairidx = src_p // 2       # row in the pair-packed table, < 25600

    key = (core * TILES + tile_of) * 2 + par
    counts = np.bincount(key, minlength=NCORES * TILES * 2).reshape(-1)
    BA = max(int(np.ceil(counts.max() / 128)), 1)
    NBT = 2 * BA               # blocks per tile (even + odd regions)
    NBLK = TILES * NBT
    SLOTS = NBLK * 128
    assert BA <= 15, BA

    chunk_tiles = [list(range(t, min(t + TPC, TILES)))
                   for t in range(0, TILES, TPC)]

    # slot layout: per chunk, [tiles x BA even blocks][tiles x BA odd blocks]
    slot_base = {}
    s0 = 0
    for tl in chunk_tiles:
        for g in (0, 1):
            for ti in tl:
                slot_base[(ti, g)] = s0
                s0 += BA * 128
    assert s0 == SLOTS

    order = np.argsort(key, kind="stable")
    ends = np.cumsum(counts)
    starts = ends - counts
    rank = np.empty(E, np.int64)
    rank[order] = np.arange(E) - starts[key[order]]
    base_arr = np.zeros((NCORES, TILES, 2), np.int64)
    for ti in range(TILES):
        for g in range(2):
            base_arr[:, ti, g] = slot_base[(ti, g)]
    slot_of_edge = base_arr.reshape(-1)[key] + rank

    gidx = np.zeros((NCORES, 128, SLOTS // 16), np.int16)
    ohm = np.zeros((NCORES, 128, NBLK, 128), np.uint8)  # fp8 bits via view
    one_fp8 = np.float32(1.0).astype(FP8).view(np.uint8)
    easl = np.zeros((NCORES, SLOTS, EF), np.float32)
    for c in range(NCORES):
        m = core == c
        sl = slot_of_edge[m]
        ohm[c][sl % 128, sl // 128, drel[m]] = one_fp8
        easl[c][sl] = edge_attr[m]
        iv = np.zeros(SLOTS, np.int64)
        iv[sl] = pairidx[m]
        j = np.arange(SLOTS)
        gidx[c][j % 16, j // 16] = iv.astype(np.int16)
        gidx[c] = np.tile(gidx[c][:16], (8, 1))
    ohm = ohm.view(FP8)

    # eaT2 stacked pairing: unit u pairs (even block k, odd block k) of a
    # chunk; top rows = even block's ea, bottom = odd block's.
    eaT2 = np.zeros((NCORES, 2 * EF, SLOTS // 2), BF16)
    for ci, tl in enumerate(chunk_tiles):
        ntl = len(tl)
        nbE = ntl * BA
        sc = slot_base[(tl[0], 0)]          # chunk start slot
        u0 = sc // 256                       # first unit (128 cols each)
        L = nbE * 128
        ev = easl[:, sc:sc + L, :]                      # [C, L, EF]
        od = easl[:, sc + L:sc + 2 * L, :]
        # unit k cols [128k,128k+128) <- even block k's 128 slots
        eaT2[:, :EF, u0 * 128:(u0 + nbE) * 128] = \
            ev.transpose(0, 2, 1).astype(BF16)
        eaT2[:, EF:, u0 * 128:(u0 + nbE) * 128] = \
            od.transpose(0, 2, 1).astype(BF16)

    # node-side tensors
    xpad = np.zeros((N, DIM), np.float32)
    xpad[:, :XF] = x
    T1 = np.zeros((NPAD, DIM), np.float32)
    flat = slot2node.reshape(-1)
    valid = flat >= 0
    T1[valid] = xpad[flat[valid]]
    t0 = T1.reshape(NPAIR, 2 * DIM).astype(BF16)

    hT0 = np.zeros((NCORES, DIM, NLOC), BF16)
    poh = np.zeros((NCORES, 128, TILES, GWIN, 128), np.uint8)
    for c in range(NCORES):
        sn = slot2node[c]
        v = sn >= 0
        hT0[c][:, v.nonzero()[0]] = xpad[sn[v]].T
        gid = np.full(NLOC, -1, np.int64)
        gid[v] = batch[sn[v]]
        sli = np.nonzero(v)[0]
        gv = gid[sli]
        poh[c][sli % 128, sli // 128, gv // 128, gv % 128] = one_fp8
    poh = poh.view(FP8)

    # per-graph inverse counts (global, host-known): graph g=128w+j -> [j, w]
    cnt = np.bincount(batch, minlength=G).astype(np.float32)
    invc = (1.0 / np.maximum(cnt, 1.0)).reshape(GWIN, 128).T.copy()

    def padw(a, r, cc):
        out = np.zeros((r, cc), np.float32)
        a = np.asarray(a, np.float32)
        out[:a.shape[0], :a.shape[1]] = a
        return out

    # fold BN scale g' = g/sqrt(v+eps) into m2 (g' > 0 so relu commutes);
    # the BN shift b' = b - m*g' is applied as a plain tensor add.
    bn_gp, bn_bp = [], []
    for i in range(1, 5):
        g_ = np.asarray(inputs[f"bn{i}_g"], np.float32)
        b_ = np.asarray(inputs[f"bn{i}_b"], np.float32)
        m_ = np.asarray(inputs[f"bn{i}_m"], np.float32)
        v_ = np.asarray(inputs[f"bn{i}_v"], np.float32)
        gp = g_ / np.sqrt(v_ + 1e-5)
        assert (gp > 0).all(), "BN scale must be positive to fold through relu"
        bn_gp.append(gp)
        bn_bp.append(b_ - m_ * gp)

    wb = {}
    for li, p in ((1, "c1"), (2, "c2"), (3, "c3")):
        be1 = padw(inputs[f"{p}_be1"], EF, DIM)
        be2 = padw(inputs[f"{p}_be2"], DIM, DIM)
        be1_2 = np.zeros((2 * EF, 128), np.float32)
        be1_2[:EF, :DIM] = be1
        be1_2[EF:, DIM:] = be1
        be2_2 = np.zeros((128, 128), np.float32)
        be2_2[:DIM, :DIM] = be2
        be2_2[DIM:, DIM:] = be2
        wb[f"be1_{li}"] = be1_2.astype(BF16)
        wb[f"be2_{li}"] = be2_2.astype(BF16)
        wb[f"m1_{li}"] = padw(inputs[f"{p}_m1"], DIM, DIM).astype(BF16)
    # m2 is folded with the layer's BN scale -> one copy per LAYER (not conv)
    for l in range(NLAYERS):
        p = ("c1", "c2", "c3", "c3")[l]
        m2 = padw(inputs[f"{p}_m2"], DIM, DIM) * bn_gp[l][None, :]
        wb[f"m2f_{l}"] = m2.astype(BF16)
        wb[f"bt_{l}"] = bn_bp[l].reshape(DIM, 1)
    eps = [float(np.asarray(inputs[f"{p}_eps"]).reshape(-1)[0])
           for p in ("c1", "c2", "c3")]
    epsv = np.array([[eps[0]], [eps[1]], [eps[2]], [eps[2]]], np.float32)

    common = {
        "t0": t0,
        "idf32": np.eye(128, dtype=np.float32),
        "idbf": np.eye(128, dtype=np.float32).astype(BF16),
        "invc": invc,
        "epsv": epsv,
        "fc1_w": np.asarray(inputs["fc1_w"], np.float32).reshape(2, 128, DIM)
                   .transpose(1, 0, 2).copy(),
        "fc1_b": np.asarray(inputs["fc1_b"], np.float32).reshape(DIM, 1),
        "fc2_w": np.asarray(inputs["fc2_w"], np.float32),
        "fc2_b": np.asarray(inputs["fc2_b"], np.float32).reshape(DIM, 1),
        "fc3_w": np.asarray(inputs["fc3_w"], np.float32),
        "fc3_b": np.asarray(inputs["fc3_b"], np.float32).reshape(DIM, 1),
        "fc4_w": np.asarray(inputs["fc4_w"], np.float32),
        "fc4_b": np.asarray(inputs["fc4_b"], np.float32).reshape(1, 1),
    }
    common.update(wb)

    in_maps = []
    for c in range(NCORES):
        m = dict(common)
        m["gidx"] = gidx[c]
        m["ohm"] = ohm[c]
        m["eaT2"] = eaT2[c]
        m["hT0"] = hT0[c]
        m["poh"] = poh[c]
        in_maps.append(m)

    struct = dict(BA=BA, NBT=NBT, NBLK=NBLK, SLOTS=SLOTS,
                  chunk_tiles=chunk_tiles, slot_base=slot_base)
    return in_maps, struct


# ------------------------------------------------------------- bass program --

def _build(struct):
    from concourse import bacc, tile, mybir
    f32, bf16, i16 = mybir.dt.float32, mybir.dt.bfloat16, mybir.dt.int16
    fp8 = mybir.dt.float8e4
    Alu = mybir.AluOpType
    Act = mybir.ActivationFunctionType

    BA, NBT = struct["BA"], struct["NBT"]
    NBLK, SLOTS = struct["NBLK"], struct["SLOTS"]
    chunk_tiles = struct["chunk_tiles"]
    NB = TPC * NBT            # max blocks per chunk

    nc = bacc.Bacc("TRN2", target_bir_lowering=False, debug=False,
                   num_devices=NCORES, num_swdge_queues=4)

    def din(name, shape, dt=f32):
        return nc.dram_tensor(name, shape, dt, kind="ExternalInput")

    t0 = din("t0", [NPAIR, 2 * DIM], bf16)
    gidx_d = din("gidx", [128, SLOTS // 16], i16)
    ohm_d = din("ohm", [128, NBLK, 128], fp8)
    eaT2_d = din("eaT2", [2 * EF, SLOTS // 2], bf16)
    hT0_d = din("hT0", [DIM, NLOC], bf16)
    poh_d = din("poh", [128, TILES, GWIN, 128], fp8)
    idf32_d = din("idf32", [128, 128])
    idbf_d = din("idbf", [128, 128], bf16)
    invc_d = din("invc", [128, GWIN])
    epsv_d = din("epsv", [4, 1])
    wdict = {}
    for li in (1, 2, 3):
        wdict[f"be1_{li}"] = din(f"be1_{li}", [2 * EF, 128], bf16)
        wdict[f"be2_{li}"] = din(f"be2_{li}", [128, 128], bf16)
        wdict[f"m1_{li}"] = din(f"m1_{li}", [DIM, DIM], bf16)
    for l in range(NLAYERS):
        wdict[f"m2f_{l}"] = din(f"m2f_{l}", [DIM, DIM], bf16)
        wdict[f"bt_{l}"] = din(f"bt_{l}", [DIM, 1])
    fc1_w = din("fc1_w", [128, 2, DIM])
    fc2_w = din("fc2_w", [DIM, DIM])
    fc3_w = din("fc3_w", [DIM, DIM])
    fc4_w = din("fc4_w", [DIM, 1])
    fcb_d = {"b1": din("fc1_b", [DIM, 1]), "b2": din("fc2_b", [DIM, 1]),
             "b3": din("fc3_b", [DIM, 1]), "b4": din("fc4_b", [1, 1])}

    out_d = nc.dram_tensor("out", [1, G], f32, kind="ExternalOutput")
    _dbg = os.environ.get("BASSGIN_DEBUG", "0") == "1"
    if _dbg:
        dbg_hT = [nc.dram_tensor(f"dbg_hT{l}", [DIM, NLOC], f32,
                                 kind="ExternalOutput") for l in range(NLAYERS)]
        dbg_xnk = nc.dram_tensor("dbg_xnk", [128, TILES, 4 * DIM], f32,
                                 kind="ExternalOutput")
        dbg_btb = nc.dram_tensor("dbg_btb", [DIM, NLAYERS, 512], f32,
                                 kind="ExternalOutput")
        dbg_z = nc.dram_tensor("dbg_z", [DIM, 512], f32, kind="ExternalOutput")
        dbg_msg = nc.dram_tensor("dbg_msg", [128, TPC * NBT, DIM], f32,
                                 kind="ExternalOutput")
        dbg_arin = nc.dram_tensor("dbg_arin", [128, GWIN, 256], f32,
                                  kind="ExternalOutput")
        dbg_pf = nc.dram_tensor("dbg_pf", [128, GWIN, 256], f32,
                                kind="ExternalOutput")
        dbg_pT = nc.dram_tensor("dbg_pT", [128, 2, 512], f32,
                                kind="ExternalOutput")
    bounce = [nc.dram_tensor(f"bounce{l}", [NLOC, DIM], bf16)
              for l in range(NLAYERS - 1)]
    tables = [t0] + [nc.dram_tensor(f"T{l}", [NPAIR, 2 * DIM], bf16,
                                    addr_space="Shared")
                     for l in (1, 2, 3)]
    arin_d = nc.dram_tensor("arin", [128, GWIN, 256], f32)
    arout_d = nc.dram_tensor("arout", [128, GWIN, 256], f32, addr_space="Shared")

    qctr = [0]

    def next_q():
        q = qctr[0] % 4
        qctr[0] += 1
        return q

    with tile.TileContext(nc) as tc:
        with tc.tile_pool(name="res", bufs=1) as res, \
             tc.tile_pool(name="hsrcp", bufs=3) as hsrcp, \
             tc.tile_pool(name="msgp", bufs=2) as msgp, \
             tc.tile_pool(name="e1p", bufs=2) as e1p, \
             tc.tile_pool(name="eap", bufs=2) as eap, \
             tc.tile_pool(name="ohp", bufs=2) as ohp, \
             tc.tile_pool(name="smallp", bufs=2) as smallp, \
             tc.tile_pool(name="psA", bufs=2, space="PSUM") as psA, \
             tc.tile_pool(name="psB", bufs=1, space="PSUM") as psB, \
             tc.tile_pool(name="psP", bufs=2, space="PSUM") as psP, \
             tc.tile_pool(name="psG", bufs=1, space="PSUM") as psG, \
             tc.tile_pool(name="psX", bufs=1, space="PSUM") as psX:

            # ---------------- residents
            def load(name, shape, dt, dram):
                tl_ = res.tile(shape, dt, tag=name)
                nc.sync.dma_start(out=tl_[:], in_=dram[:])
                return tl_

            idf_sb = load("idf", [128, 128], f32, idf32_d)
            idb_sb = load("idb", [128, 128], bf16, idbf_d)
            invc_sb = load("invc", [128, GWIN], f32, invc_d)
            w_sb = {k: load(f"w_{k}", list(d.shape), d.dtype, d)
                    for k, d in wdict.items()}
            fc1w_sb = load("fc1w", [128, 2, DIM], f32, fc1_w)
            fc2w_sb = load("fc2w", [DIM, DIM], f32, fc2_w)
            fc3w_sb = load("fc3w", [DIM, DIM], f32, fc3_w)
            fc4w_sb = load("fc4w", [DIM, 1], f32, fc4_w)
            fcb_sb = {k: load(f"fcb{k}", list(d.shape), f32, d)
                      for k, d in fcb_d.items()}
            hT = load("hT", [DIM, NLOC], bf16, hT0_d)
            # per-layer natural-layout node tiles, kept for pooling
            xnk = res.tile([128, TILES, 4 * DIM], bf16, tag="xnk")

            # eps broadcast [64,1] per layer: (1+eps)
            eps1p = []
            for l in range(NLAYERS):
                e0 = res.tile([1, 1], f32, tag=f"eps0_{l}")
                nc.sync.dma_start(out=e0[:], in_=epsv_d[l:l + 1, :])
                eb = res.tile([DIM, 1], f32, tag=f"epsb{l}")
                nc.gpsimd.partition_broadcast(eb[:], e0[:], channels=DIM)
                e1 = res.tile([DIM, 1], f32, tag=f"eps1p{l}")
                nc.vector.tensor_scalar_add(e1[:], eb[:], 1.0)
                eps1p.append(e1)

            # bn shift b' broadcast to [64, 512] (scale is folded into m2)
            btb = []
            for l in range(NLAYERS):
                bb = res.tile([DIM, 512], bf16, tag=f"btb{l}")
                nc.vector.memset(bb[:], 0.0)
                nc.vector.tensor_scalar_add(bb[:], bb[:], w_sb[f"bt_{l}"][:])
                btb.append(bb)
                if _dbg:
                    nc.gpsimd.dma_start(out=dbg_btb[:, l, :], in_=bb[:])

            # ---------------- layers
            arin_sb = res.tile([128, GWIN, 256], f32, tag="arin")
            nc.vector.memset(arin_sb[:], 0.0)
            for l in range(NLAYERS):
                wl = min(l + 1, 3)
                be1 = w_sb[f"be1_{wl}"]; be2 = w_sb[f"be2_{wl}"]
                m1 = w_sb[f"m1_{wl}"]; m2 = w_sb[f"m2f_{l}"]
                tbl = tables[l]

                for ci, tl in enumerate(chunk_tiles):
                    ntl = len(tl)
                    nbE = ntl * BA          # even-parity blocks in chunk
                    nb = 2 * nbE
                    ncols = nb * 64         # bond-encoder edge columns
                    s0 = struct["slot_base"][(tl[0], 0)]
                    gb0 = s0 // 128         # first global block of chunk
                    # gather idx staging + 4 gathers on 4 SWDGE queues
                    gix = smallp.tile([128, TPC * NBT * 8], i16, tag="gix")
                    nc.sync.dma_start(out=gix[:, 0:nb * 8],
                                      in_=gidx_d[:, s0 // 16:(s0 + nb * 128) // 16])
                    hsrc = hsrcp.tile([128, NB, 2 * DIM], bf16, tag="hsrc")
                    # one gather per (parity, tile); chunks alternate SWDGE
                    # queue pairs so consecutive chunks never contend for
                    # descriptor-ring space (the ring holds ~4K descriptors
                    # per queue; same-queue back-to-back gathers stall the
                    # prep in await_space until the prior one drains)
                    qbase = (ci % 2) * 2
                    for g in (0, 1):
                        for k in range(ntl):
                            ba = g * nbE + k * BA
                            nc.gpsimd.dma_gather(
                                out_ap=hsrc[:, ba:ba + BA, :],
                                in_ap=tbl[0:NPAIR, :],
                                idxs_ap=gix[:, ba * 8:(ba + BA) * 8],
                                num_idxs=BA * 128, num_idxs_reg=BA * 128,
                                elem_size=2 * DIM,
                                single_packet=False,
                                queue_num=qbase + (k % 2))

                    # bond encoder stage 1 for this chunk
                    c0 = s0 // 2
                    ea_sb = eap.tile([2 * EF, TPC * NBT * 64], bf16, tag="ea")
                    nc.sync.dma_start(out=ea_sb[:, 0:ncols],
                                      in_=eaT2_d[:, c0:c0 + ncols])
                    e1t = e1p.tile([128, TPC * NBT * 64], bf16, tag="e1")
                    g0 = 0
                    while g0 < ncols:
                        gw = min(512, ncols - g0)
                        ps1 = psA.tile([128, 512], f32, tag="ps1")
                        nc.tensor.matmul(ps1[:, 0:gw], be1[:], ea_sb[:, g0:g0 + gw],
                                         start=True, stop=True)
                        nc.scalar.activation(e1t[:, g0:g0 + gw], ps1[:, 0:gw],
                                             Act.Relu)
                        g0 += gw

                    # scatter one-hot stream for this chunk
                    oh_t = ohp.tile([128, NB, 128], fp8, tag="oh")
                    nc.sync.dma_start(out=oh_t[:, 0:nb, :],
                                      in_=ohm_d[:, gb0:gb0 + nb, :])

                    # stage 2: unit j pairs (even blk j, odd blk j); psum
                    # [128 edge-cols, 2, 64] per unit. msg = relu(hsrc+e).
                    msg = msgp.tile([128, NB, DIM], bf16, tag="msg")
                    for b0 in range(0, nbE, 4):
                        un = min(4, nbE - b0)
                        pse = psP.tile([128, 4, 2, DIM], f32, tag="pse")
                        for j in range(un):
                            u = b0 + j
                            nc.tensor.matmul(pse[:, j, :, :],
                                             e1t[:, 128 * u:128 * (u + 1)],
                                             be2[:], start=True, stop=True)
                        nc.vector.tensor_add(
                            msg[:, b0:b0 + un, :],
                            hsrc[:, b0:b0 + un, 0:DIM], pse[:, 0:un, 0, :])
                        nc.vector.tensor_add(
                            msg[:, nbE + b0:nbE + b0 + un, :],
                            hsrc[:, nbE + b0:nbE + b0 + un, DIM:2 * DIM],
                            pse[:, 0:un, 1, :])
                        nc.scalar.activation(msg[:, b0:b0 + un, :],
                                             msg[:, b0:b0 + un, :], Act.Relu)
                        nc.scalar.activation(msg[:, nbE + b0:nbE + b0 + un, :],
                                             msg[:, nbE + b0:nbE + b0 + un, :],
                                             Act.Relu)

                    # scatter: per tile, accumulate its blocks into PSUM
                    aggps = psG.tile([DIM, 512], f32, tag="aggps")
                    for k, ti in enumerate(tl):
                        blocks = ([k * BA + i for i in range(BA)] +
                                  [nbE + k * BA + i for i in range(BA)])
                        for j, b in enumerate(blocks):
                            nc.tensor.matmul(aggps[:, 128 * k:128 * (k + 1)],
                                             msg[:, b, :], oh_t[:, b, :],
                                             start=(j == 0), stop=(j == NBT - 1))

                    # ---- node update for this chunk's tiles (bf16 MLP)
                    gw = ntl * 128
                    sl = slice(128 * tl[0], 128 * tl[0] + gw)
                    zT = smallp.tile([DIM, 512], bf16, tag="zr")
                    nc.vector.scalar_tensor_tensor(zT[:, 0:gw], hT[:, sl],
                                                   eps1p[l][:], aggps[:, 0:gw],
                                                   Alu.mult, Alu.add)
                    if _dbg and l == 0 and ci == 0:
                        nc.gpsimd.dma_start(out=dbg_z[:], in_=zT[:])
                        nc.gpsimd.dma_start(out=dbg_msg[:, 0:nb, :],
                                            in_=msg[:, 0:nb, :])
                    ps1 = psA.tile([128, 512], f32, tag="ps1")
                    nc.tensor.matmul(ps1[0:DIM, 0:gw], m1[:], zT[:, 0:gw],
                                     start=True, stop=True)
                    r1 = smallp.tile([DIM, 512], bf16, tag="zr")
                    nc.scalar.activation(r1[:, 0:gw], ps1[0:DIM, 0:gw], Act.Relu)
                    ps2 = psB.tile([128, 512], f32, tag="ps2")
                    nc.tensor.matmul(ps2[0:DIM, 0:gw], m2[:], r1[:, 0:gw],
                                     start=True, stop=True)
                    rr = smallp.tile([DIM, 512], bf16, tag="rr")
                    nc.scalar.activation(rr[:, 0:gw], ps2[0:DIM, 0:gw], Act.Relu)
                    nc.vector.tensor_add(hT[:, sl], rr[:, 0:gw],
                                         btb[l][:, 0:gw])
                    for k, ti in enumerate(tl):
                        pst = psP.tile([128, 2, DIM], bf16, tag="pse", name="pst")
                        nc.tensor.transpose(pst[:, 0, :],
                                            hT[:, 128 * ti:128 * (ti + 1)],
                                            idb_sb[0:DIM, 0:DIM])
                        nc.vector.tensor_copy(xnk[:, ti, 64 * l:64 * (l + 1)],
                                              pst[:, 0, :])
                        if l < NLAYERS - 1:
                            nc.sync.dma_start(
                                out=bounce[l][128 * ti:128 * (ti + 1), :],
                                in_=xnk[:, ti, 64 * l:64 * (l + 1)])
                    if l == NLAYERS - 1:
                        # pooling for this chunk's tiles: chunk-local PSUM
                        # chains, accumulated across chunks in SBUF
                        poh_t = smallp.tile([128, TPC, GWIN, 128], fp8,
                                            tag="poht")
                        t00 = tl[0]
                        nc.sync.dma_start(out=poh_t[:, 0:ntl, :, :],
                                          in_=poh_d[:, t00:t00 + ntl, :, :])
                        pxa = psX.tile([128, 2, 256], f32, tag="pl01",
                                       name="pxa")
                        pxb = psX.tile([128, 2, 256], f32, tag="pl23",
                                       name="pxb")
                        px = [pxa, pxb]
                        for w in range(GWIN):
                            for k in range(ntl):
                                nc.tensor.matmul(
                                    px[w // 2][:, w % 2, :],
                                    poh_t[:, k, w, :],
                                    xnk[:, t00 + k, 0:256],
                                    start=(k == 0), stop=(k == ntl - 1))
                        nc.vector.tensor_add(arin_sb[:, 0:2, :],
                                             arin_sb[:, 0:2, :], pxa[:])
                        nc.vector.tensor_add(arin_sb[:, 2:4, :],
                                             arin_sb[:, 2:4, :], pxb[:])

                if _dbg:
                    nc.gpsimd.dma_start(out=dbg_hT[l][:], in_=hT[:])
                if l < NLAYERS - 1:
                    nc.gpsimd.collective_compute(
                        "AllGather", Alu.bypass,
                        replica_groups=[list(range(NCORES))],
                        ins=[bounce[l][:]], outs=[tables[l + 1][:]])

            # ---------------- pooling tail: AllReduce + mean + head
            if _dbg:
                nc.gpsimd.dma_start(out=dbg_xnk[:], in_=xnk[:])
            nc.sync.dma_start(out=arin_d[:], in_=arin_sb[:])
            if _dbg:
                nc.sync.dma_start(out=dbg_arin[:], in_=arin_sb[:])
            nc.gpsimd.collective_compute(
                "AllReduce", Alu.add, replica_groups=[list(range(NCORES))],
                ins=[arin_d[:]], outs=[arout_d[:]])
            pf = res.tile([128, GWIN, 256], f32, tag="pf")
            nc.sync.dma_start(out=pf[:], in_=arout_d[:])
            if _dbg:
                nc.sync.dma_start(out=dbg_pf[:], in_=pf[:])

            # mean (host-computed inverse counts) + head
            pT = res.tile([128, 2, 512], f32, tag="pT")
            for w in range(GWIN):
                pm = smallp.tile([128, 256], f32, tag="pm")
                nc.vector.tensor_scalar_mul(pm[:], pf[:, w, :],
                                            invc_sb[:, w:w + 1])
                for k in range(2):
                    pst = psP.tile([128, 512], f32, tag="pse", name="hpst")
                    nc.tensor.transpose(pst[:, 0:128],
                                        pm[:, 128 * k:128 * (k + 1)],
                                        idf_sb[:])
                    nc.vector.tensor_copy(pT[:, k, 128 * w:128 * (w + 1)],
                                          pst[:, 0:128])
            if _dbg:
                nc.sync.dma_start(out=dbg_pT[:], in_=pT[:])
            hps = psA.tile([128, 512], f32, tag="ps1")
            for k in range(2):
                nc.tensor.matmul(hps[0:DIM, :], fc1w_sb[:, k, :], pT[:, k, :],
                                 start=(k == 0), stop=(k == 1))
            h1 = res.tile([DIM, 512], f32, tag="h1")
            nc.scalar.activation(h1[:], hps[0:DIM, :], Act.Relu,
                                 bias=fcb_sb["b1"][:])
            hps2 = psB.tile([128, 512], f32, tag="ps2")
            nc.tensor.matmul(hps2[0:DIM, :], fc2w_sb[:], h1[:], start=True, stop=True)
            h2 = res.tile([DIM, 512], f32, tag="h2")
            nc.scalar.activation(h2[:], hps2[0:DIM, :], Act.Relu,
                                 bias=fcb_sb["b2"][:])
            hps3 = psA.tile([128, 512], f32, tag="ps1")
            nc.tensor.matmul(hps3[0:DIM, :], fc3w_sb[:], h2[:], start=True, stop=True)
            h3 = res.tile([DIM, 512], f32, tag="h3")
            nc.scalar.activation(h3[:], hps3[0:DIM, :], Act.Relu,
                                 bias=fcb_sb["b3"][:])
            hps4 = psB.tile([128, 512], f32, tag="ps2")
            nc.tensor.matmul(hps4[0:1, :], fc4w_sb[:], h3[:], start=True, stop=True)
            ho = res.tile([1, G], f32, tag="ho")
            nc.scalar.activation(ho[:], hps4[0:1, :], Act.Identity,
                                 bias=fcb_sb["b4"][:])
            nc.sync.dma_start(out=out_d[:], in_=ho[:])

    nc.compile()
    return nc


# ------------------------------------------------------------------ runner --

_CACHE = {}


def kernel(**inputs):
    from concourse.bass_utils import run_bass_kernel_spmd
    in_maps, struct = _prep(inputs)
    key = (struct["BA"], os.environ.get("BASSGIN_DEBUG", "0"))
    if key not in _CACHE:
        _CACHE[key] = _build(struct)
    nc = _CACHE[key]
    trace = os.environ.get("BASSGIN_TRACE", "0") == "1"
    res = run_bass_kernel_spmd(nc, in_maps, core_ids=list(range(NCORES)),
                               trace=trace)
    kernel.last_result = res
    out = res.results[0]["out"].reshape(G).astype(np.float32)
    return out
